# revision 33
# baseline (speedup 1.0000x reference)
"""Trainium2 Bass kernel for nn_BottleneckBit (ResNet bottleneck with ternary-
quantized convs + BN + SiLU + residual).

Strategy (v2):
- Data-parallel over batch: 64 images -> 8 cores x 8 images.
- conv1 (1x1, 1024->256): bf16 matmuls, x streamed bp-major (2-image groups)
  so each group's k-chain finishes early and its SiLU/ACT pipelines out.
- conv2 (3x3, 256->256): fp8e4m3 DoubleRow matmuls (K=256 per instruction,
  2x PE throughput; measured 101ns/MM vs 166ns bf16 at half the work).
  h1 is stored fp8 in a zero-padded 16x16 layout [p, k2, img, 256]; weights
  [p, k2, 128] per (j, tap). 9-tap accumulation chains per image, tap-outer /
  image-inner so the LDWEIGHTS of the next tap hides under the current MMs.
  Two images share one PSUM bank (cols 0:196 / 196:392) so one 392-wide ACT
  drains both.
- conv3 (1x1, 256->1024): bf16, residual added on the PE via identity matmul
  (3-MM chains), 2-bank PSUM tiles so one strided ACT drains 2 output tiles.
- Ternary weight trick: wq = clip(round(w/s),-1,1)*s is exact in fp8/bf16;
  per-channel scale and BN fold into ACT scale/bias (a3 folded into w3).
- Precision budget: only h1 is fp8 (sim: rel err 1.27e-2 vs 2e-2 gate);
  x, h2, out stay bf16.
"""
import numpy as np
import ml_dtypes

import concourse.bass as bass
import concourse.mybir as mybir
from concourse import bacc
from concourse.tile import TileContext
from concourse.bass_utils import run_bass_kernel_spmd
from concourse.masks import make_identity

BN_EPS = 1e-5
Q_EPS = 1e-8

B, CIN, H, W = 64, 1024, 14, 14
WIDTH, COUT = 256, 1024
N_CORES = 8
BC = B // N_CORES          # images per core = 8
PIX = H * W                # 196
P = 128
CIN_T = CIN // P           # 8
W_T = WIDTH // P           # 2
COUT_T = COUT // P         # 8
IPG = 2                    # images per L1/L3 matmul group
BP = BC // IPG             # 4 image-pair groups per core
NN = IPG * PIX             # 392
HP, WP = H + 2, W + 2      # 16
PADPIX = HP * WP           # 256

_F32 = mybir.dt.float32
_BF16 = mybir.dt.bfloat16
_FP8 = mybir.dt.float8e4
_AF = mybir.ActivationFunctionType
_DR = mybir.MatmulPerfMode.DoubleRow


def build(act_func=None):
    if act_func is None:
        act_func = _AF.Silu
    nc = bacc.Bacc()

    # x bp-major: [p, bp, k, i, pix] = [p, bp*8*392 + k*392 + n]
    xd = nc.declare_dram_parameter("x", [P, BP * CIN_T * NN], _BF16, isOutput=False)
    w1d = nc.declare_dram_parameter("w1", [P, CIN_T * WIDTH], _FP8, isOutput=False)
    # w2: [p, j, tap, k2, m] fp8 (DoubleRow pairs)
    w2d = nc.declare_dram_parameter("w2", [P, W_T * 9 * WIDTH], _FP8, isOutput=False)
    w3d = nc.declare_dram_parameter("w3", [P, W_T * COUT], _BF16, isOutput=False)
    ccd = nc.declare_dram_parameter("cc", [P, 8 + COUT_T], _F32, isOutput=False)
    # out folded: [p, j*BC*PIX + bp*392 + n]
    outd = nc.declare_dram_parameter("out", [P, COUT_T * BC * PIX], _BF16, isOutput=True)

    with TileContext(nc) as tc:
        with tc.tile_pool(name="weights", bufs=1) as wpool, \
             tc.tile_pool(name="acts", bufs=1) as apool, \
             tc.tile_pool(name="outs", bufs=4) as opool, \
             tc.tile_pool(name="ps1", bufs=2, space="PSUM") as ps1pool, \
             tc.tile_pool(name="ps2", bufs=3, space="PSUM") as ps2pool:

            # ---- early gpsimd work: wsrc memset (unblocks PE warmup), then
            # weight DMAs + h1pad halo memsets on the gpsimd queue ----
            wsrc = apool.tile([P, P], _BF16, name="wsrc")
            nc.gpsimd.memset(wsrc[:, :], 0.0)
            # w1 on the scalar queue, issued before the Silu table load
            w1t = wpool.tile([P, CIN_T * WIDTH], _FP8, name="w1t")
            nc.scalar.dma_start(out=w1t[:, :], in_=w1d[:, :])
            # dummy ACT so walrus schedules the Silu table load during the
            # boot idle window; uses its own scratch tile so the warmup
            # matmuls (which read wsrc) don't serialize behind it
            scr = apool.tile([P, 1], _F32, name="scr")
            nc.scalar.activation(scr[:, 0:1], scr[:, 0:1], act_func)

            # x: bp0 k-pair chunks split across sync+gpsimd (fast early
            # semaphores); bp1 4-way, bp2/bp3 halves on sync so each chunk
            # lands just ahead of its L1 phase
            xt = apool.tile([P, BP * CIN_T * NN], _BF16, name="xt")
            for k2 in range(CIN_T // 2):
                lo = k2 * 2 * NN
                q = nc.sync if k2 % 2 == 0 else nc.gpsimd
                q.dma_start(out=xt[:, lo:lo + 2 * NN],
                            in_=xd[:, lo:lo + 2 * NN])

            # PE warmup: start the HAM busy-window during the DMA lead-in,
            # sized to hand off seamlessly to L1 (a gap resets the 3.4us HAM
            # activity window and L1 then runs at 1.2GHz)
            wps = ps1pool.tile([P, 512], _F32, name="wps", tag="ps")
            for _ in range(36):
                nc.tensor.matmul(wps[:, 0:P], wsrc[:, :], wsrc[:, :],
                                 start=True, stop=True)

            cct = wpool.tile([P, 8 + COUT_T], _F32, name="cct")
            nc.gpsimd.dma_start(out=cct[:, :], in_=ccd[:, :])
            for k2 in range(CIN_T // 2):   # bp1 4-way on sync
                lo = CIN_T * NN + k2 * 2 * NN
                nc.sync.dma_start(out=xt[:, lo:lo + 2 * NN],
                                  in_=xd[:, lo:lo + 2 * NN])
            for bp in (2, 3):              # bp2/bp3 halves on sync
                for h in range(2):
                    lo = bp * CIN_T * NN + h * (CIN_T // 2) * NN
                    nc.sync.dma_start(out=xt[:, lo:lo + (CIN_T // 2) * NN],
                                      in_=xd[:, lo:lo + (CIN_T // 2) * NN])

            # padded h1 (fp8): [p, img, k2, 16, 16] (img-major so halo memsets
            # split by image wave)
            identt = wpool.tile([P, P], _BF16, name="identt")
            make_identity(nc, identt[:, :])
            h1p = apool.tile([P, BC * W_T * PADPIX], _FP8, name="h1p")
            nc.gpsimd.memset(h1p[:, 0:BC * W_T * PADPIX // 2], 0.0)
            w2t = wpool.tile([P, W_T * 9 * WIDTH], _FP8, name="w2t")
            nc.gpsimd.dma_start(out=w2t[:, :], in_=w2d[:, :])
            nc.gpsimd.memset(h1p[:, BC * W_T * PADPIX // 2:], 0.0)
            w3t = wpool.tile([P, W_T * COUT], _BF16, name="w3t")
            nc.gpsimd.dma_start(out=w3t[:, :], in_=w3d[:, :])

            h1p5 = h1p.rearrange("p (i k r c) -> p i k r c",
                                 i=BC, k=W_T, r=HP, c=WP)
            w2r = w2t.rearrange("p (j t k m) -> p j t k m", j=W_T, t=9, k=W_T)
            # h2 (bf16): [p, k2, img, 196]
            h2 = apool.tile([P, W_T * BC * PIX], _BF16, name="h2")
            h2r = h2.rearrange("p (k i n) -> p k i n", k=W_T, i=BC)

            def xs(bp, k):
                lo = bp * CIN_T * NN + k * NN
                return xt[:, lo:lo + NN]

            # ---- L1 per bp: j-outer k-chains so j0's ACT overlaps j1's MMs ----
            def l1(bp, pool=None):
                pool = pool or ps1pool
                for j in range(W_T):
                    ps = pool.tile([P, 512], _F32, name=f"ps1_{bp}_{j}",
                                   tag="ps" if pool is ps1pool else "ps3")
                    for k in range(CIN_T):
                        nc.tensor.matmul(
                            ps[:, 0:NN],
                            w1t[:, k * WIDTH + j * P: k * WIDTH + (j + 1) * P],
                            xs(bp, k), start=(k == 0), stop=(k == CIN_T - 1))
                    src = ps[:, 0:NN].rearrange("p (i r c) -> p i r c",
                                                i=IPG, r=H, c=W)
                    dst = h1p5[:, IPG * bp:IPG * (bp + 1), j, 1:1 + H, 1:1 + W]
                    nc.scalar.activation(dst, src, act_func,
                                         bias=cct[:, 2 + j:3 + j],
                                         scale=cct[:, 0 + j:1 + j])

            # ---- L2 per bp (2 imgs): DoubleRow; the 2 imgs share a bank ----
            def l2w(bp):
                i0 = IPG * bp
                ps = [ps1pool.tile([P, 512], _F32, name=f"ps2_{bp}_{j}",
                                   tag="ps") for j in range(W_T)]
                for j in range(W_T):
                    for tap in range(9):
                        dy, dx = tap // 3, tap % 3
                        for di in range(IPG):
                            rhs = h1p5[:, i0 + di, :, dy:dy + H, dx:dx + W]
                            out = ps[j][:, di * PIX:(di + 1) * PIX]
                            # start only on the FIRST MM touching the bank:
                            # start=True pends-zero the whole 2KB bank, so the
                            # second image's chain must not re-issue it (its
                            # first write auto-zeroes via the pending mark).
                            nc.tensor.matmul(out, w2r[:, j, tap], rhs,
                                             start=(tap == 0 and di == 0),
                                             stop=(tap == 8),
                                             perf_mode=_DR,
                                             skip_group_check=(di == 1))
                for j in range(W_T):
                    nc.scalar.activation(
                        h2r[:, j, i0:i0 + IPG, :],
                        ps[j][:, 0:NN], act_func,
                        bias=cct[:, 6 + j:7 + j], scale=cct[:, 4 + j:5 + j])

            # ---- L3 per bp: bf16 conv; j-pairs share a 2-bank psum tile so
            # one strided vector add (+x', c3 pre-folded into x on host) and
            # one strided ACT (silu, bias=0) drain 2 output tiles ----
            def l3b(bp, pe_resid=False):
                for jq in range(COUT_T // 4):
                    ot = opool.tile([P, 4 * NN], _BF16, name="ot", tag="ot")
                    for jp2 in range(2):
                        jp = 2 * jq + jp2
                        pst = ps2pool.tile([P, 1024], _F32,
                                           name=f"ps3_{bp}_{jp}", tag="ps3")
                        for dj in range(2):
                            j = 2 * jp + dj
                            out = pst[:, dj * 512:dj * 512 + NN]
                            for k in range(W_T):
                                nc.tensor.matmul(
                                    out,
                                    w3t[:, k * COUT + j * P:
                                        k * COUT + (j + 1) * P],
                                    h2r[:, k, IPG * bp:IPG * (bp + 1), :],
                                    start=(k == 0),
                                    stop=(not pe_resid and k == W_T - 1))
                            if pe_resid:
                                nc.tensor.matmul(out, identt[:, :],
                                                 xs(bp, j), start=False,
                                                 stop=True)
                        psv = pst.rearrange("p (d n) -> p d n", d=2)[:, :, 0:NN]
                        if not pe_resid:
                            xv = xt[:, bp * CIN_T * NN + 2 * jp * NN:
                                    bp * CIN_T * NN + (2 * jp + 2) * NN]
                            xvr = xv.rearrange("p (d n) -> p d n", d=2)
                            nc.vector.tensor_add(out=psv, in0=psv, in1=xvr)
                        dst = ot.rearrange("p (q d n) -> p q d n",
                                           q=2, d=2)[:, jp2]
                        nc.scalar.activation(dst, psv, act_func, bias=0.0,
                                             scale=1.0)
                    # one DMA per 4 output-channel tiles (per 2 in fine
                    # mode, used for the last bp to shorten the tail chain)
                    odr = outd.rearrange("p (j n) -> p j n", j=COUT_T)
                    osr = ot.rearrange("p (j n) -> p j n", j=4)
                    nc.sync.dma_start(
                        out=odr[:, 4 * jq:4 * jq + 4,
                                bp * NN:(bp + 1) * NN], in_=osr)

            l1(0, pool=ps2pool)
            l1(1, pool=ps2pool)
            l2w(0)
            l1(2, pool=ps2pool)
            l3b(0)
            l2w(1)
            l1(3, pool=ps2pool)
            l3b(1)
            l2w(2)
            l3b(2)
            l2w(3)
            l3b(3, pe_resid=True)

    nc.finalize()
    return nc


def _prep_host(x, w1, b1, g1, be1, m1, v1,
               w2, b2, g2, be2, m2, v2,
               w3, b3, g3, be3, m3, v3):
    def quant(w):
        w = np.asarray(w, np.float32)
        s = np.median(np.abs(w).reshape(w.shape[0], -1), axis=1)
        s = np.maximum(s, np.float32(Q_EPS)).astype(np.float32)
        t = np.clip(np.round(w / s[:, None, None, None]), -1.0, 1.0).astype(np.float32)
        return t, s

    def fold(s, b, g, be, m, v):
        sc = np.asarray(g, np.float64) / np.sqrt(np.asarray(v, np.float64) + BN_EPS)
        a = (np.asarray(s, np.float64) * sc).astype(np.float32)
        c = (np.asarray(b, np.float64) * sc + np.asarray(be, np.float64)
             - np.asarray(m, np.float64) * sc).astype(np.float32)
        return a, c

    t1, s1 = quant(w1)
    t2, s2 = quant(w2)
    t3, s3 = quant(w3)
    a1, c1 = fold(s1, b1, g1, be1, m1, v1)
    a2, c2 = fold(s2, b2, g2, be2, m2, v2)
    a3, c3 = fold(s3, b3, g3, be3, m3, v3)

    bf = ml_dtypes.bfloat16
    fp8 = ml_dtypes.float8_e4m3

    def part_fold(m2d):
        kk, mm = m2d.shape
        return np.ascontiguousarray(
            m2d.reshape(kk // P, P, mm).transpose(1, 0, 2).reshape(P, -1))

    # w1: [p, k*WIDTH + j*128 + m] = t1[j*128+m, k*128+p]
    w1_dev = part_fold(t1[:, :, 0, 0].T).astype(fp8)
    # w2: [p, (((j*9)+tap)*2+k2)*128 + m] = t2[j*128+m, k2*128+p, dy, dx]
    w2_host = np.zeros((P, W_T * 9 * WIDTH), np.float32)
    t2r = t2.reshape(W_T, P, W_T, P, 3, 3)  # [j, m, k2, p, dy, dx]
    for j in range(W_T):
        for tap in range(9):
            dy, dx = tap // 3, tap % 3
            blk = t2r[j, :, :, :, dy, dx]          # [m, k2, p]
            w2_host[:, ((j * 9 + tap) * W_T) * P:
                    ((j * 9 + tap) * W_T + W_T) * P] = (
                np.ascontiguousarray(blk.transpose(2, 1, 0)).reshape(P, -1))
    w2_dev = w2_host.astype(fp8)
    # w3 with a3 folded (bf16): [p, k*COUT + j*128 + m]
    w3_dev = part_fold((t3[:, :, 0, 0] * a3[:, None]).T).astype(bf)

    # c3 rides on x (x' = x + c3 per channel) so the L3 epilogue needs no
    # per-j bias; conv1 sees the constant shift, corrected exactly in c1:
    # c1' = c1 - a1 * (t1 @ c3)
    k1 = t1[:, :, 0, 0].astype(np.float64) @ np.asarray(c3, np.float64)
    c1 = (c1.astype(np.float64) - a1.astype(np.float64) * k1).astype(np.float32)

    cc = np.zeros((P, 8 + COUT_T), np.float32)
    cc[:, 0:2] = a1.reshape(W_T, P).T
    cc[:, 2:4] = c1.reshape(W_T, P).T
    cc[:, 4:6] = a2.reshape(W_T, P).T
    cc[:, 6:8] = c2.reshape(W_T, P).T

    const = {"w1": w1_dev, "w2": w2_dev, "w3": w3_dev,
             "cc": np.ascontiguousarray(cc)}

    x = np.asarray(x, np.float32) + np.asarray(c3, np.float32)[None, :, None, None]
    in_maps = []
    for c in range(N_CORES):
        # [p, bp*CIN_T*NN + k*NN + i*PIX + n]; row k*128+p of channel dim
        xc = x[c * BC:(c + 1) * BC].reshape(BP, IPG, CIN_T, P, PIX)
        xc = xc.transpose(3, 0, 2, 1, 4).reshape(P, -1)  # p, bp, k, i, pix
        in_maps.append({"x": np.ascontiguousarray(xc).astype(bf), **const})
    return in_maps


def _run(inputs, trace=False, act_func=None, **spmd_kwargs):
    nc = build(act_func)
    in_maps = _prep_host(**inputs)
    res = run_bass_kernel_spmd(nc, in_maps, list(range(N_CORES)),
                               trace=trace, **spmd_kwargs)
    outs = []
    for c in range(N_CORES):
        of = res.results[c]["out"].astype(np.float32)  # [p, j*BC*PIX + bp*NN + n]
        oc = of.reshape(P, COUT_T, BP, IPG, PIX)
        # channel j*128+p, image bp*2+i
        oc = oc.transpose(2, 3, 1, 0, 4).reshape(BC, COUT, H, W)
        outs.append(oc)
    full = np.concatenate(outs, axis=0).astype(np.float32)
    return full, res


def kernel(**inputs):
    out, _ = _run(inputs)
    return out
